# revision 5
# baseline (speedup 1.0000x reference)
"""Single-head causal attention on 8 TRN2 NeuronCores.

Problem shapes (hardcoded): B=8, T=2048, C=1024, H=64, fp32 I/O.
    q = x @ Wq; k = x @ Wk; v = x @ Wv          (per batch element)
    wei = softmax(causal_mask(q @ k.T * C**-0.5))
    out = wei @ v

Sharding: pure data parallel — one batch element per core, no collectives.

Per-core algorithm (all matmuls bf16 with fp32 PSUM accumulation):
  - host pre-transposes x -> xT [C, T] so C (the contraction dim of the
    QKV projections) lands on SBUF partitions; host packs [Wq|Wk].
  - qkT[128, T] = [Wq|Wk].T @ x.T computed as one matmul stream
    (lhsT = wqk C-block [128,128], rhs = xT [128,512]).  qT = rows 0:64,
    kT = rows 64:128; kT is moved to partitions 0:64 by an SBUF->SBUF DMA
    so the S^T matmul operands share a partition range.
  - vT[64, T] likewise from Wv; v_nat [T,H] recovered with identity
    matmuls (out = vT_blk.T @ I64), stored with an appended ones column:
    v1 = [v | 1].
  - S^T[Tk, Tq] = k q^T per (128 Tk x 512 Tq) tile via
    lhsT=kT block [64,128], rhs=qT [64,512]; only causal tiles computed.
  - P^T = exp(S^T / 32) on ScalarE (no max subtraction needed: logits
    have std ~0.25, so exp never overflows; softmax is shift invariant).
    Diagonal 128x128 blocks multiplied by a 0/1 causal mask.
  - [out | denom]^T [65, Tq] accumulated as lhsT=v1 [128,65],
    rhs=P^T [128,512]; the ones column yields softmax denominators for
    free in row 64.
  - epilogue: transpose 128-col blocks back to natural layout with an
    fp32 identity matmul, then out = num * (1/denom) per partition (DVE)
    and DMA to DRAM as fp32 [T, H].
"""

import os
import numpy as np
import ml_dtypes

import concourse.bass as bass
import concourse.mybir as mybir
import concourse.tile as tile
from concourse import bacc
from concourse.bass_utils import run_bass_kernel_spmd

B, T, C, H = 8, 2048, 1024, 64
NCB = C // 128          # 8 C-blocks
NT = T // 128           # 16 T-blocks of 128
NJ = T // 512           # 4 T-blocks of 512
SCALE = float(C) ** -0.5  # 1/32

BF16 = mybir.dt.bfloat16
F32 = mybir.dt.float32
npbf16 = ml_dtypes.bfloat16


def build_attention(nc: bass.Bass, tc: tile.TileContext, ctx):
    xT_d = nc.dram_tensor("xT", [C, T], BF16, kind="ExternalInput").ap()
    wqk_d = nc.dram_tensor("wqk", [C, 128], BF16, kind="ExternalInput").ap()
    wv_d = nc.dram_tensor("wv", [C, H], BF16, kind="ExternalInput").ap()
    out_d = nc.dram_tensor("out", [T, H], F32, kind="ExternalOutput").ap()

    ident64 = nc.inline_tensor(np.eye(64, dtype=npbf16), name="ident64").ap()
    ident65 = nc.inline_tensor(np.eye(65, dtype=np.float32), name="ident65").ap()
    causal_np = np.triu(np.ones((128, 128), dtype=npbf16))  # keep where Tk<=Tq
    causal_d = nc.inline_tensor(causal_np, name="causal").ap()

    consts = ctx.enter_context(tc.tile_pool(name="consts", bufs=1))
    xts = ctx.enter_context(tc.tile_pool(name="xts", bufs=3))
    persist = ctx.enter_context(tc.tile_pool(name="persist", bufs=1))
    pts = ctx.enter_context(tc.tile_pool(name="pts", bufs=4))
    outts = ctx.enter_context(tc.tile_pool(name="outts", bufs=2))
    outs = ctx.enter_context(tc.tile_pool(name="outs", bufs=3))
    smalls = ctx.enter_context(tc.tile_pool(name="smalls", bufs=2))

    i64_sb = consts.tile([64, 64], BF16, tag="i64")
    nc.sync.dma_start(out=i64_sb, in_=ident64)
    i65_sb = consts.tile([65, 65], F32, tag="i65")
    nc.sync.dma_start(out=i65_sb, in_=ident65)
    causal_sb = consts.tile([128, 128], BF16, tag="causal")
    nc.sync.dma_start(out=causal_sb, in_=causal_d)

    # weights: wqk [C,128] -> [128, NCB, 128] (partition = C%128)
    wqk_sb = consts.tile([128, NCB, 128], BF16, tag="wqk")
    nc.sync.dma_start(out=wqk_sb, in_=wqk_d.rearrange("(c p) h -> p c h", p=128))
    wv_sb = consts.tile([128, NCB, H], BF16, tag="wv")
    nc.sync.dma_start(out=wv_sb, in_=wv_d.rearrange("(c p) h -> p c h", p=128))

    # persistent SBUF tensors
    qkT = persist.tile([128, T], BF16, tag="qkT")       # rows 0:64 qT, 64:128 kT
    kT = persist.tile([64, T], BF16, tag="kT")          # kT at partitions 0:64
    vT = persist.tile([64, T], BF16, tag="vT")
    v1 = persist.tile([128, NT, H + 1], BF16, tag="v1")  # [v | 1] per Tk block
    nc.gpsimd.memset(v1, 1.0)  # ones column; cols 0:H overwritten below

    # ---- Phase 1: projections --------------------------------------------
    # PSUM budget: 8 accumulator banks (4 qk + 4 vT) live through the c loop;
    # this pool closes before the phase-2 pools open so banks are reused.
    with tc.tile_pool(name="ps_proj", bufs=4, space="PSUM") as ps_proj:
        qk_ps = [ps_proj.tile([128, 512], F32, tag="qkp", name=f"qk_ps{j}")
                 for j in range(NJ)]
        vT_ps = [ps_proj.tile([64, 512], F32, tag="vtp", name=f"vT_ps{j}")
                 for j in range(NJ)]
        for c in range(NCB):
            xt = xts.tile([128, T], BF16, tag="xt")
            nc.sync.dma_start(out=xt, in_=xT_d[c * 128:(c + 1) * 128, :])
            for j in range(NJ):
                nc.tensor.matmul(
                    qk_ps[j], lhsT=wqk_sb[:, c, :],
                    rhs=xt[:, j * 512:(j + 1) * 512],
                    start=(c == 0), stop=(c == NCB - 1))
            for j in range(NJ):
                nc.tensor.matmul(
                    vT_ps[j], lhsT=wv_sb[:, c, :],
                    rhs=xt[:, j * 512:(j + 1) * 512],
                    start=(c == 0), stop=(c == NCB - 1))
        for j in range(NJ):
            sl = slice(j * 512, (j + 1) * 512)
            nc.vector.tensor_copy(qkT[:, sl], qk_ps[j])
            # move kT half down to partitions 0:64 (align with qT for S^T mm)
            nc.sync.dma_start(out=kT[:, sl], in_=qkT[64:128, sl])
            nc.vector.tensor_copy(vT[:, sl], vT_ps[j])

    with (
        tc.tile_pool(name="ps_big", bufs=3, space="PSUM") as ps_big,
        tc.tile_pool(name="ps_acc", bufs=2, space="PSUM") as ps_acc,
        tc.tile_pool(name="ps_sm", bufs=2, space="PSUM") as ps_sm,
    ):
        # v natural [Tk,H] via identity matmul: v_blk = vT_blk.T @ I64
        for t in range(NT):
            vp = ps_sm.tile([128, H + 1], F32, tag="sm")
            nc.tensor.matmul(vp[:, 0:H], lhsT=vT[:, t * 128:(t + 1) * 128],
                             rhs=i64_sb, start=True, stop=True)
            nc.vector.tensor_copy(v1[:, t, 0:H], vp[:, 0:H])

        # ---- Phase 2: attention ------------------------------------------
        for j in range(NJ):
            av = ps_acc.tile([65, 512], F32, tag="acc")
            nblk = 4 * j + 4
            for i in range(nblk):
                g = i - 4 * j             # >=0: diagonal-band tile
                n0 = max(0, g) * 128      # first valid column in this tile
                sp = ps_big.tile([128, 512], F32, tag="big")
                nc.tensor.matmul(
                    sp[:, n0:512],
                    lhsT=kT[:, i * 128:(i + 1) * 128],
                    rhs=qkT[0:64, j * 512 + n0:(j + 1) * 512],
                    start=True, stop=True)
                pt = pts.tile([128, 512], BF16, tag="pt")
                nc.scalar.activation(
                    pt[:, n0:512], sp[:, n0:512],
                    mybir.ActivationFunctionType.Exp, scale=SCALE)
                if g >= 0:  # mask upper triangle of the diagonal block
                    nc.vector.tensor_mul(
                        pt[:, n0:n0 + 128], pt[:, n0:n0 + 128], causal_sb)
                nc.tensor.matmul(
                    av[:, n0:512], lhsT=v1[:, i, :], rhs=pt[:, n0:512],
                    start=(i == 0), stop=(i == nblk - 1))
            # epilogue: back to natural layout + normalize
            osb = outts.tile([65, 512], F32, tag="osb")
            nc.vector.tensor_copy(osb, av)
            for t in range(4):
                op = ps_sm.tile([128, H + 1], F32, tag="sm")
                nc.tensor.matmul(op, lhsT=osb[:, t * 128:(t + 1) * 128],
                                 rhs=i65_sb, start=True, stop=True)
                rc = smalls.tile([128, 1], F32, tag="rc")
                nc.vector.reciprocal(rc, op[:, H:H + 1])
                ot = outs.tile([128, H], F32, tag="ot")
                nc.vector.tensor_scalar_mul(ot, op[:, 0:H], rc)
                r0 = (j * 4 + t) * 128
                nc.sync.dma_start(out=out_d[r0:r0 + 128, :], in_=ot)


_CACHED = {}


def _get_nc():
    if "nc" not in _CACHED:
        from contextlib import ExitStack
        nc = bacc.Bacc("TRN2", target_bir_lowering=False, debug=False,
                       num_devices=B)
        with tile.TileContext(nc) as tc:
            with ExitStack() as ctx:
                build_attention(nc, tc, ctx)
        nc.compile()
        _CACHED["nc"] = nc
    return _CACHED["nc"]


def kernel(inputs, Wq, Wk, Wv):
    inputs = np.asarray(inputs, dtype=np.float32)
    wqk = np.concatenate([np.asarray(Wq), np.asarray(Wk)], axis=1)
    wqk = np.ascontiguousarray(wqk, dtype=npbf16)
    wv = np.ascontiguousarray(np.asarray(Wv), dtype=npbf16)

    in_maps = []
    for b in range(B):
        xT = np.ascontiguousarray(inputs[b].T).astype(npbf16)
        in_maps.append({"xT": xT, "wqk": wqk, "wv": wv})

    nc = _get_nc()
    res = run_bass_kernel_spmd(nc, in_maps, core_ids=list(range(B)))
    out = np.stack([res.results[b]["out"] for b in range(B)], axis=0)
    return out.astype(np.float32)


# revision 6
# speedup vs baseline: 1.3408x; 1.3408x over previous
"""Single-head causal attention on 8 TRN2 NeuronCores.

Problem shapes (hardcoded): B=8, T=2048, C=1024, H=64, fp32 I/O.
    q = x @ Wq; k = x @ Wk; v = x @ Wv          (per batch element)
    wei = softmax(causal_mask(q @ k.T * C**-0.5))
    out = wei @ v

Sharding: pure data parallel — one batch element per core, no collectives.

Per-core algorithm (matmuls bf16, fp32 PSUM accumulation):
  - host pre-transposes x -> xT [C, T] so C (the contraction dim of the
    QKV projections) lands on SBUF partitions; host packs [Wq|Wk].
  - per 512-wide Tq slice j: qkT = [Wq|Wk].T @ xT (qT rows 0:64, kT rows
    64:128); SBUF->SBUF DMAs build qT2 = [qT;qT] (both partition halves)
    and kT2 (Tk blocks 2m/2m+1 stacked in partition halves) so S^T
    matmuls can run ROW-PACKED: two K=64 matmuls concurrently in the two
    64-row halves of the PE array.
  - vT = Wv.T @ xT; v natural recovered by identity matmuls, stored as
    v1 = [v | 1] (ones column -> softmax denominators for free).
  - S^T[Tk,Tq] tiles -> P^T = exp(S^T/32) on ScalarE (no max subtraction
    needed: logits std ~0.25; softmax is shift invariant). Diagonal
    128x128 blocks masked by a 0/1 causal mask; fully-masked column
    ranges are simply never computed (restricted-N matmuls).
  - [out|denom]^T accumulated via lhsT=v1 [128,65], rhs=P^T; epilogue
    transposes 128-col blocks back to natural layout with an identity
    matmul and normalizes per partition (reciprocal + tensor_scalar).
  - projections and attention are interleaved per j so the PE never
    idles >3.4us (keeps the HAM clock gate at K=8/8); epilogue of slice
    j is emitted after the projections of slice j+1 so sem-waits never
    sit at the head of the PE FIFO.
"""

import numpy as np
import ml_dtypes

import concourse.bass as bass
import concourse.mybir as mybir
import concourse.tile as tile
from concourse import bacc
from concourse.bass_utils import run_bass_kernel_spmd

B, T, C, H = 8, 2048, 1024, 64
NCB = C // 128          # 8 C-blocks
NT = T // 128           # 16 Tk-blocks of 128
NJ = T // 512           # 4 Tq-slices of 512
SCALE = float(C) ** -0.5  # 1/32

BF16 = mybir.dt.bfloat16
F32 = mybir.dt.float32
npbf16 = ml_dtypes.bfloat16


def build_attention(nc: bass.Bass, tc: tile.TileContext, ctx):
    from contextlib import ExitStack  # noqa

    xT_d = nc.dram_tensor("xT", [C, T], BF16, kind="ExternalInput").ap()
    wqk_d = nc.dram_tensor("wqk", [C, 128], BF16, kind="ExternalInput").ap()
    wv_d = nc.dram_tensor("wv", [C, H], BF16, kind="ExternalInput").ap()
    out_d = nc.dram_tensor("out", [T, H], F32, kind="ExternalOutput").ap()

    ident64 = nc.inline_tensor(np.eye(64, dtype=npbf16), name="ident64").ap()
    ident65 = nc.inline_tensor(np.eye(65, dtype=npbf16), name="ident65").ap()
    causal_np = np.triu(np.ones((128, 128), dtype=npbf16))  # keep Tk<=Tq
    causal_d = nc.inline_tensor(causal_np, name="causal").ap()

    consts = ctx.enter_context(tc.tile_pool(name="consts", bufs=1))
    xts = ctx.enter_context(tc.tile_pool(name="xts", bufs=18))
    persist = ctx.enter_context(tc.tile_pool(name="persist", bufs=1))
    pts = ctx.enter_context(tc.tile_pool(name="pts", bufs=4))
    outts = ctx.enter_context(tc.tile_pool(name="outts", bufs=2))
    outs = ctx.enter_context(tc.tile_pool(name="outs", bufs=3))
    smalls = ctx.enter_context(tc.tile_pool(name="smalls", bufs=2))
    ps_qk = ctx.enter_context(tc.tile_pool(name="ps_qk", bufs=1, space="PSUM"))
    ps_vt = ctx.enter_context(tc.tile_pool(name="ps_vt", bufs=1, space="PSUM"))
    ps_big = ctx.enter_context(tc.tile_pool(name="ps_big", bufs=4, space="PSUM"))
    ps_acc = ctx.enter_context(tc.tile_pool(name="ps_acc", bufs=2, space="PSUM"))

    i64_sb = consts.tile([64, 64], BF16, tag="i64")
    nc.sync.dma_start(out=i64_sb, in_=ident64)
    i65_sb = consts.tile([65, 65], BF16, tag="i65")
    nc.sync.dma_start(out=i65_sb, in_=ident65)
    causal_sb = consts.tile([128, 128], BF16, tag="causal")
    nc.sync.dma_start(out=causal_sb, in_=causal_d)
    wqk_sb = consts.tile([128, NCB, 128], BF16, tag="wqk")
    nc.sync.dma_start(out=wqk_sb, in_=wqk_d.rearrange("(c p) h -> p c h", p=128))
    wv_sb = consts.tile([128, NCB, H], BF16, tag="wv")
    nc.sync.dma_start(out=wv_sb, in_=wv_d.rearrange("(c p) h -> p c h", p=128))

    qkT = persist.tile([128, T], BF16, tag="qkT")
    qT2 = persist.tile([128, T], BF16, tag="qT2")    # [qT; qT]
    kT2 = persist.tile([128, T // 2], BF16, tag="kT2")  # Tk pairs in halves
    vT = persist.tile([64, T], BF16, tag="vT")
    v1 = persist.tile([128, NT, H + 1], BF16, tag="v1")  # [v | 1]
    nc.gpsimd.memset(v1, 1.0)

    # state carried between j iterations for deferred emission
    pending_av = None     # (av_psum, j) awaiting epilogue
    for j in range(NJ):
        jsl = slice(j * 512, (j + 1) * 512)

        # ---- projections for slice j --------------------------------
        xtj = []
        for c in range(NCB):
            xt = xts.tile([128, 512], BF16, tag="xt", name=f"xt{c}_{j}")
            nc.sync.dma_start(out=xt, in_=xT_d[c * 128:(c + 1) * 128, jsl])
            xtj.append(xt)
        qk_ps = ps_qk.tile([128, 512], F32, tag="qkp", name=f"qk_ps{j}")
        for c in range(NCB):
            nc.tensor.matmul(qk_ps, lhsT=wqk_sb[:, c, :], rhs=xtj[c],
                             start=(c == 0), stop=(c == NCB - 1))
        nc.vector.tensor_copy(qkT[:, jsl], qk_ps)
        # replicate qT into both partition halves; restack kT blocks
        nc.sync.dma_start(out=qT2[0:64, jsl], in_=qkT[0:64, jsl])
        nc.sync.dma_start(out=qT2[64:128, jsl], in_=qkT[0:64, jsl])
        for b in range(4):  # Tk block 4j+b -> kT2 half b%2, col block 2j+b//2
            half = (b % 2) * 64
            c0 = j * 256 + (b // 2) * 128
            nc.sync.dma_start(
                out=kT2[half:half + 64, c0:c0 + 128],
                in_=qkT[64:128, j * 512 + b * 128:j * 512 + (b + 1) * 128])

        vT_ps = ps_vt.tile([64, 512], F32, tag="vtp", name=f"vT_ps{j}")
        for c in range(NCB):
            nc.tensor.matmul(vT_ps, lhsT=wv_sb[:, c, :], rhs=xtj[c],
                             start=(c == 0), stop=(c == NCB - 1))
        nc.vector.tensor_copy(vT[:, jsl], vT_ps)
        for t in range(4 * j, 4 * j + 4):  # v natural blocks for slice j
            vp = ps_acc.tile([128, H + 1], F32, tag="accsm", name=f"vp{t}")
            nc.tensor.matmul(vp[:, 0:H], lhsT=vT[:, t * 128:(t + 1) * 128],
                             rhs=i64_sb, start=True, stop=True)
            nc.vector.tensor_copy(v1[:, t, 0:H], vp[:, 0:H])

        # ---- deferred epilogue of slice j-1 -------------------------
        if pending_av is not None:
            emit_epilogue(nc, outts, outs, smalls, ps_acc, i65_sb, out_d,
                          *pending_av)
            pending_av = None

        # ---- attention for slice j (row-packed S^T, pipelined AV) ---
        av = ps_acc.tile([65, 512], F32, tag="accsm", name=f"av{j}")
        nblk = 4 * j + 4
        prev = None  # (pt_tiles, n0s, i0)
        for m in range(2 * j + 2):
            pt_pair, n0_pair = [], []
            for half_idx, i in ((0, 2 * m), (1, 2 * m + 1)):
                g = i - 4 * j
                n0 = max(0, g) * 128
                p0 = half_idx * 64
                sp = ps_big.tile([128, 512], F32, tag="big", name=f"sp{j}_{i}")
                nc.tensor.matmul(
                    sp[:, n0:512],
                    lhsT=kT2[p0:p0 + 64, m * 128:(m + 1) * 128],
                    rhs=qT2[p0:p0 + 64, j * 512 + n0:(j + 1) * 512],
                    start=True, stop=True)
                pt = pts.tile([128, 512], BF16, tag="pt", name=f"pt{j}_{i}")
                nc.scalar.activation(
                    pt[:, n0:512], sp[:, n0:512],
                    mybir.ActivationFunctionType.Exp, scale=SCALE)
                if g >= 0:
                    nc.vector.tensor_mul(
                        pt[:, n0:n0 + 128], pt[:, n0:n0 + 128], causal_sb)
                pt_pair.append(pt)
                n0_pair.append(n0)
            if prev is not None:
                emit_av(nc, av, v1, *prev, nblk)
            prev = (pt_pair, n0_pair, 2 * m)
        emit_av(nc, av, v1, *prev, nblk)
        pending_av = (av, j)

    emit_epilogue(nc, outts, outs, smalls, ps_acc, i65_sb, out_d, *pending_av)


def emit_av(nc, av, v1, pt_pair, n0_pair, i0, nblk):
    for d in range(2):
        i = i0 + d
        n0 = n0_pair[d]
        nc.tensor.matmul(av[:, n0:512], lhsT=v1[:, i, :],
                         rhs=pt_pair[d][:, n0:512],
                         start=(i == 0), stop=(i == nblk - 1))


def emit_epilogue(nc, outts, outs, smalls, ps_acc, i65_sb, out_d, av, j):
    osb = outts.tile([65, 512], BF16, tag="osb", name=f"osb{j}")
    nc.scalar.copy(osb, av)  # ScalarE: f32 PSUM -> bf16 SBUF
    for t in range(4):
        op = ps_acc.tile([128, H + 1], F32, tag="accsm", name=f"op{j}_{t}")
        nc.tensor.matmul(op, lhsT=osb[:, t * 128:(t + 1) * 128], rhs=i65_sb,
                         start=True, stop=True)
        rc = smalls.tile([128, 1], F32, tag="rc", name=f"rc{j}_{t}")
        nc.vector.reciprocal(rc, op[:, H:H + 1])
        ot = outs.tile([128, H], F32, tag="ot", name=f"ot{j}_{t}")
        nc.vector.tensor_scalar_mul(ot, op[:, 0:H], rc)
        r0 = (j * 4 + t) * 128
        nc.sync.dma_start(out=out_d[r0:r0 + 128, :], in_=ot)


_CACHED = {}


def _get_nc():
    if "nc" not in _CACHED:
        from contextlib import ExitStack
        nc = bacc.Bacc("TRN2", target_bir_lowering=False, debug=False,
                       num_devices=B)
        with tile.TileContext(nc) as tc:
            with ExitStack() as ctx:
                build_attention(nc, tc, ctx)
        nc.compile()
        _CACHED["nc"] = nc
    return _CACHED["nc"]


def kernel(inputs, Wq, Wk, Wv):
    inputs = np.asarray(inputs, dtype=np.float32)
    wqk = np.concatenate([np.asarray(Wq), np.asarray(Wk)], axis=1)
    wqk = np.ascontiguousarray(wqk).astype(npbf16)
    wv = np.ascontiguousarray(np.asarray(Wv)).astype(npbf16)

    in_maps = []
    for b in range(B):
        xT = np.ascontiguousarray(inputs[b].T).astype(npbf16)
        in_maps.append({"xT": xT, "wqk": wqk, "wv": wv})

    nc = _get_nc()
    res = run_bass_kernel_spmd(nc, in_maps, core_ids=list(range(B)))
    out = np.stack([res.results[b]["out"] for b in range(B)], axis=0)
    return out.astype(np.float32)


# revision 8
# speedup vs baseline: 1.3569x; 1.0120x over previous
"""Single-head causal attention on 8 TRN2 NeuronCores.

Problem shapes (hardcoded): B=8, T=2048, C=1024, H=64, fp32 I/O.
    q = x @ Wq; k = x @ Wk; v = x @ Wv          (per batch element)
    wei = softmax(causal_mask(q @ k.T * C**-0.5))
    out = wei @ v

Sharding: pure data parallel — one batch element per core, no collectives.

Per-core algorithm (matmuls bf16, fp32 PSUM accumulation):
  - host pre-transposes x -> xT [C, T] so C (the contraction dim of the
    QKV projections) lands on SBUF partitions; host packs [Wq|Wk].
  - per 512-wide Tq slice j: qkT = [Wq|Wk].T @ xT (qT rows 0:64, kT rows
    64:128); SBUF->SBUF DMAs build qT2 = [qT;qT] (both partition halves)
    and kT2 (Tk blocks 2m/2m+1 stacked in partition halves) so S^T
    matmuls run ROW-PACKED: two K=64 matmuls execute concurrently in the
    two 64-row halves of the PE array (row_grp packing).
  - vT = Wv.T @ xT; v natural recovered by row-packed identity matmuls,
    stored as v1 = [v | 1] (ones column -> softmax denominators free).
  - S^T pair tiles share one [128,1024] PSUM tensor (2 banks) so the
    exp runs as a single wide ScalarE ACTIVATE where possible.
    P^T = exp(S^T/32); no max subtraction needed (logits std ~0.25;
    softmax is shift invariant). Diagonal 128x128 blocks multiplied by a
    0/1 causal mask; fully-masked column ranges never computed
    (restricted-N matmuls).
  - [out|denom]^T accumulated via lhsT=v1 [128,65], rhs=P^T; epilogue
    transposes 128-col blocks back to natural layout with an identity
    matmul and normalizes per partition (reciprocal + tensor_scalar).
  - projections and attention interleave per j so the PE never idles
    (HAM clock gate stays released); slice j's epilogue is emitted after
    slice j+1's projections so ACT-dependent matmuls never stall the PE
    FIFO head; DMAs are spread across the Sync/GpSimd/Vector queues.
"""

import numpy as np
import ml_dtypes

import concourse.bass as bass
import concourse.mybir as mybir
import concourse.tile as tile
from concourse import bacc
from concourse.bass_utils import run_bass_kernel_spmd

B, T, C, H = 8, 2048, 1024, 64
NCB = C // 128          # 8 C-blocks
NT = T // 128           # 16 Tk-blocks of 128
NJ = T // 512           # 4 Tq-slices of 512
SCALE = float(C) ** -0.5  # 1/32

BF16 = mybir.dt.bfloat16
F32 = mybir.dt.float32
npbf16 = ml_dtypes.bfloat16


def build_attention(nc: bass.Bass, tc: tile.TileContext, ctx):
    xT_d = nc.dram_tensor("xT", [C, T], BF16, kind="ExternalInput").ap()
    wqk_d = nc.dram_tensor("wqk", [C, 128], BF16, kind="ExternalInput").ap()
    wv_d = nc.dram_tensor("wv", [C, H], BF16, kind="ExternalInput").ap()
    out_d = nc.dram_tensor("out", [T, H], F32, kind="ExternalOutput").ap()

    i64_2_np = np.concatenate([np.eye(64, dtype=npbf16)] * 2, axis=0)
    ident64_2 = nc.inline_tensor(i64_2_np, name="ident64_2").ap()
    ident65 = nc.inline_tensor(np.eye(65, dtype=npbf16), name="ident65").ap()
    causal_np = np.triu(np.ones((128, 128), dtype=npbf16))  # keep Tk<=Tq
    causal_d = nc.inline_tensor(causal_np, name="causal").ap()

    consts = ctx.enter_context(tc.tile_pool(name="consts", bufs=1))
    xts = ctx.enter_context(tc.tile_pool(name="xts", bufs=18))
    persist = ctx.enter_context(tc.tile_pool(name="persist", bufs=1))
    pts = ctx.enter_context(tc.tile_pool(name="pts", bufs=4))
    outts = ctx.enter_context(tc.tile_pool(name="outts", bufs=2))
    outs = ctx.enter_context(tc.tile_pool(name="outs", bufs=3))
    smalls = ctx.enter_context(tc.tile_pool(name="smalls", bufs=2))
    ps_qk = ctx.enter_context(tc.tile_pool(name="ps_qk", bufs=1, space="PSUM"))
    ps_vt = ctx.enter_context(tc.tile_pool(name="ps_vt", bufs=1, space="PSUM"))
    ps_big = ctx.enter_context(tc.tile_pool(name="ps_big", bufs=2, space="PSUM"))
    ps_acc = ctx.enter_context(tc.tile_pool(name="ps_acc", bufs=2, space="PSUM"))

    # consts on the GpSimd queue so the Sync queue starts xt loads at once
    i64_sb = consts.tile([128, 64], BF16, tag="i64")
    nc.gpsimd.dma_start(out=i64_sb, in_=ident64_2)
    i65_sb = consts.tile([65, 65], BF16, tag="i65")
    nc.gpsimd.dma_start(out=i65_sb, in_=ident65)
    causal_sb = consts.tile([128, 128], BF16, tag="causal")
    nc.gpsimd.dma_start(out=causal_sb, in_=causal_d)
    wqk_sb = consts.tile([128, NCB, 128], BF16, tag="wqk")
    nc.gpsimd.dma_start(out=wqk_sb, in_=wqk_d.rearrange("(c p) h -> p c h", p=128))
    wv_sb = consts.tile([128, NCB, H], BF16, tag="wv")
    nc.gpsimd.dma_start(out=wv_sb, in_=wv_d.rearrange("(c p) h -> p c h", p=128))

    qkT = persist.tile([128, T], BF16, tag="qkT")
    qT2 = persist.tile([128, T], BF16, tag="qT2")       # [qT; qT]
    kT2 = persist.tile([128, T // 2], BF16, tag="kT2")  # Tk pairs in halves
    vT = persist.tile([64, T], BF16, tag="vT")
    vT2 = persist.tile([128, T // 2], BF16, tag="vT2")  # odd Tk blocks, hi half
    v1 = persist.tile([128, NT, H + 1], BF16, tag="v1")  # [v | 1]
    nc.gpsimd.memset(v1, 1.0)

    pending_av = None
    for j in range(NJ):
        jsl = slice(j * 512, (j + 1) * 512)

        # ---- projections for slice j --------------------------------
        xtj = []
        for c in range(NCB):
            xt = xts.tile([128, 512], BF16, tag="xt", name=f"xt{c}_{j}")
            eng = nc.sync if c % 2 == 0 else nc.gpsimd
            eng.dma_start(out=xt, in_=xT_d[c * 128:(c + 1) * 128, jsl])
            xtj.append(xt)
        qk_ps = ps_qk.tile([128, 512], F32, tag="qkp", name=f"qk_ps{j}")
        for c in range(NCB):
            nc.tensor.matmul(qk_ps, lhsT=wqk_sb[:, c, :], rhs=xtj[c],
                             start=(c == 0), stop=(c == NCB - 1))
        nc.vector.tensor_copy(qkT[:, jsl], qk_ps)
        # qT into both halves; kT restacked into pair layout (on DVE's
        # queue: FIFO order after the copy above comes for free)
        nc.sync.dma_start(out=qT2[0:64, jsl], in_=qkT[0:64, jsl])
        nc.sync.dma_start(out=qT2[64:128, jsl], in_=qkT[0:64, jsl])
        for b in range(4):  # Tk block 4j+b -> half b%2, col block 2j+b//2
            half = (b % 2) * 64
            c0 = j * 256 + (b // 2) * 128
            nc.gpsimd.dma_start(
                out=kT2[half:half + 64, c0:c0 + 128],
                in_=qkT[64:128, j * 512 + b * 128:j * 512 + (b + 1) * 128])

        vT_ps = ps_vt.tile([64, 512], F32, tag="vtp", name=f"vT_ps{j}")
        for c in range(NCB):
            nc.tensor.matmul(vT_ps, lhsT=wv_sb[:, c, :], rhs=xtj[c],
                             start=(c == 0), stop=(c == NCB - 1))
        nc.vector.tensor_copy(vT[:, jsl], vT_ps)
        for bb in range(2):  # odd Tk blocks 4j+1, 4j+3 -> vT2 hi half
            tb = 4 * j + 2 * bb + 1
            c0 = (2 * j + bb) * 128
            nc.gpsimd.dma_start(
                out=vT2[64:128, c0:c0 + 128],
                in_=vT[:, tb * 128:(tb + 1) * 128])
        # v natural via row-packed identity matmuls (pair of Tk blocks)
        for mt in (2 * j, 2 * j + 1):
            tA, tB = 2 * mt, 2 * mt + 1
            vpA = ps_big.tile([128, H + 1], F32, tag="big", name=f"vpA{mt}")
            vpB = ps_big.tile([128, H + 1], F32, tag="big", name=f"vpB{mt}")
            nc.tensor.matmul(vpA[:, 0:H], lhsT=vT[:, tA * 128:(tA + 1) * 128],
                             rhs=i64_sb[0:64, :], start=True, stop=True)
            nc.tensor.matmul(vpB[:, 0:H],
                             lhsT=vT2[64:128, mt * 128:(mt + 1) * 128],
                             rhs=i64_sb[64:128, :], start=True, stop=True)
            nc.vector.tensor_copy(v1[:, tA, 0:H], vpA[:, 0:H])
            nc.vector.tensor_copy(v1[:, tB, 0:H], vpB[:, 0:H])

        # ---- deferred epilogue of slice j-1 -------------------------
        if pending_av is not None:
            emit_epilogue(nc, outts, outs, smalls, ps_acc, i65_sb, out_d,
                          *pending_av)
            pending_av = None

        # ---- attention for slice j (row-packed S^T, pipelined AV) ---
        av = ps_acc.tile([65, 512], F32, tag="accsm", name=f"av{j}")
        nblk = 4 * j + 4
        prev = None
        for m in range(2 * j + 2):
            sp2 = ps_big.tile([128, 1024], F32, tag="big", name=f"sp{j}_{m}")
            pt2 = pts.tile([128, 1024], BF16, tag="pt", name=f"pt{j}_{m}")
            n0s = []
            for half_idx, i in ((0, 2 * m), (1, 2 * m + 1)):
                g = i - 4 * j
                n0 = max(0, g) * 128
                p0 = half_idx * 64
                o = half_idx * 512
                nc.tensor.matmul(
                    sp2[:, o + n0:o + 512],
                    lhsT=kT2[p0:p0 + 64, m * 128:(m + 1) * 128],
                    rhs=qT2[p0:p0 + 64, j * 512 + n0:(j + 1) * 512],
                    start=True, stop=True)
                n0s.append(n0)
            if n0s[0] == 0 and n0s[1] == 0:  # one wide exp over both banks
                nc.scalar.activation(pt2, sp2,
                                     mybir.ActivationFunctionType.Exp,
                                     scale=SCALE)
            else:
                for half_idx in range(2):
                    o, n0 = half_idx * 512, n0s[half_idx]
                    nc.scalar.activation(
                        pt2[:, o + n0:o + 512], sp2[:, o + n0:o + 512],
                        mybir.ActivationFunctionType.Exp, scale=SCALE)
            for half_idx, i in ((0, 2 * m), (1, 2 * m + 1)):
                g = i - 4 * j
                if g >= 0:  # mask upper triangle of the diagonal block
                    o = half_idx * 512 + n0s[half_idx]
                    nc.vector.tensor_mul(
                        pt2[:, o:o + 128], pt2[:, o:o + 128], causal_sb)
            if prev is not None:
                emit_av(nc, av, v1, *prev, nblk)
            prev = (pt2, n0s, 2 * m)
        emit_av(nc, av, v1, *prev, nblk)
        pending_av = (av, j)

    emit_epilogue(nc, outts, outs, smalls, ps_acc, i65_sb, out_d, *pending_av)


def emit_av(nc, av, v1, pt2, n0s, i0, nblk):
    for d in range(2):
        i = i0 + d
        o, n0 = d * 512, n0s[d]
        nc.tensor.matmul(av[:, n0:512], lhsT=v1[:, i, :],
                         rhs=pt2[:, o + n0:o + 512],
                         start=(i == 0), stop=(i == nblk - 1))


def emit_epilogue(nc, outts, outs, smalls, ps_acc, i65_sb, out_d, av, j):
    osb = outts.tile([65, 512], BF16, tag="osb", name=f"osb{j}")
    nc.scalar.copy(osb, av)  # ScalarE: f32 PSUM -> bf16 SBUF
    for t in range(4):
        op = ps_acc.tile([128, H + 1], F32, tag="accsm", name=f"op{j}_{t}")
        nc.tensor.matmul(op, lhsT=osb[:, t * 128:(t + 1) * 128], rhs=i65_sb,
                         start=True, stop=True)
        rc = smalls.tile([128, 1], F32, tag="rc", name=f"rc{j}_{t}")
        nc.vector.reciprocal(rc, op[:, H:H + 1])
        ot = outs.tile([128, H], F32, tag="ot", name=f"ot{j}_{t}")
        nc.vector.tensor_scalar_mul(ot, op[:, 0:H], rc)
        r0 = (j * 4 + t) * 128
        eng = nc.scalar if t % 2 == 0 else nc.sync
        eng.dma_start(out=out_d[r0:r0 + 128, :], in_=ot)


_CACHED = {}


def _get_nc():
    if "nc" not in _CACHED:
        from contextlib import ExitStack
        nc = bacc.Bacc("TRN2", target_bir_lowering=False, debug=False,
                       num_devices=B)
        with tile.TileContext(nc) as tc:
            with ExitStack() as ctx:
                build_attention(nc, tc, ctx)
        nc.compile()
        _CACHED["nc"] = nc
    return _CACHED["nc"]


def kernel(inputs, Wq, Wk, Wv):
    inputs = np.asarray(inputs, dtype=np.float32)
    wqk = np.concatenate([np.asarray(Wq), np.asarray(Wk)], axis=1)
    wqk = np.ascontiguousarray(wqk).astype(npbf16)
    wv = np.ascontiguousarray(np.asarray(Wv)).astype(npbf16)

    in_maps = []
    for b in range(B):
        xT = np.ascontiguousarray(inputs[b].T).astype(npbf16)
        in_maps.append({"xT": xT, "wqk": wqk, "wv": wv})

    nc = _get_nc()
    res = run_bass_kernel_spmd(nc, in_maps, core_ids=list(range(B)))
    out = np.stack([res.results[b]["out"] for b in range(B)], axis=0)
    return out.astype(np.float32)


# revision 9
# speedup vs baseline: 1.4394x; 1.0608x over previous
"""Single-head causal attention on 8 TRN2 NeuronCores.

Problem shapes (hardcoded): B=8, T=2048, C=1024, H=64, fp32 I/O.
    q = x @ Wq; k = x @ Wk; v = x @ Wv          (per batch element)
    wei = softmax(causal_mask(q @ k.T * C**-0.5))
    out = wei @ v

Sharding: pure data parallel — one batch element per core, no collectives.

Per-core algorithm (matmuls bf16, fp32 PSUM accumulation):
  - host pre-transposes x -> xT [C, T] so C (the contraction dim of the
    QKV projections) lands on SBUF partitions; host packs [Wq|Wk].
  - per 512-wide Tq slice j: qkT = [Wq|Wk].T @ xT (qT rows 0:64, kT rows
    64:128); SBUF->SBUF DMAs build qT2 = [qT;qT] (both partition halves)
    and kT2 (Tk blocks 2m/2m+1 stacked in partition halves) so S^T
    matmuls run ROW-PACKED: two K=64 matmuls execute concurrently in the
    two 64-row halves of the PE array (row_grp packing).
  - vT = Wv.T @ xT; v natural recovered by row-packed identity matmuls,
    stored as v1 = [v | 1] (ones column -> softmax denominators free).
  - S^T pair tiles share one [128,1024] PSUM tensor (2 banks) so the
    exp runs as a single wide ScalarE ACTIVATE where possible.
    P^T = exp(S^T/32); no max subtraction needed (logits std ~0.25;
    softmax is shift invariant). Diagonal 128x128 blocks multiplied by a
    0/1 causal mask; fully-masked column ranges never computed
    (restricted-N matmuls).
  - [out|denom]^T accumulated via lhsT=v1 [128,65], rhs=P^T; epilogue
    transposes 128-col blocks back to natural layout with an identity
    matmul and normalizes per partition (reciprocal + tensor_scalar).
  - projections and attention interleave per j so the PE never idles
    (HAM clock gate stays released); slice j's epilogue is emitted after
    slice j+1's projections so ACT-dependent matmuls never stall the PE
    FIFO head; DMAs are spread across the Sync/GpSimd/Vector queues.
"""

import numpy as np
import ml_dtypes

import concourse.bass as bass
import concourse.mybir as mybir
import concourse.tile as tile
from concourse import bacc
from concourse.bass_utils import run_bass_kernel_spmd

B, T, C, H = 8, 2048, 1024, 64
NCB = C // 128          # 8 C-blocks
NT = T // 128           # 16 Tk-blocks of 128
NJ = T // 512           # 4 Tq-slices of 512
SCALE = float(C) ** -0.5  # 1/32

BF16 = mybir.dt.bfloat16
F32 = mybir.dt.float32
npbf16 = ml_dtypes.bfloat16


def build_attention(nc: bass.Bass, tc: tile.TileContext, ctx):
    xT_d = nc.dram_tensor("xT", [C, T], BF16, kind="ExternalInput").ap()
    wqk_d = nc.dram_tensor("wqk", [C, 128], BF16, kind="ExternalInput").ap()
    wv_d = nc.dram_tensor("wv", [C, H], BF16, kind="ExternalInput").ap()
    out_d = nc.dram_tensor("out", [T, H], F32, kind="ExternalOutput").ap()

    i64_2_np = np.concatenate([np.eye(64, dtype=npbf16)] * 2, axis=0)
    ident64_2 = nc.inline_tensor(i64_2_np, name="ident64_2").ap()
    ident65 = nc.inline_tensor(np.eye(65, dtype=npbf16), name="ident65").ap()
    causal_np = np.triu(np.ones((128, 128), dtype=npbf16))  # keep Tk<=Tq
    causal_d = nc.inline_tensor(causal_np, name="causal").ap()

    consts = ctx.enter_context(tc.tile_pool(name="consts", bufs=1))
    xts = ctx.enter_context(tc.tile_pool(name="xts", bufs=18))
    persist = ctx.enter_context(tc.tile_pool(name="persist", bufs=1))
    pts = ctx.enter_context(tc.tile_pool(name="pts", bufs=4))
    outts = ctx.enter_context(tc.tile_pool(name="outts", bufs=2))
    outs = ctx.enter_context(tc.tile_pool(name="outs", bufs=3))
    smalls = ctx.enter_context(tc.tile_pool(name="smalls", bufs=2))
    ps_qk = ctx.enter_context(tc.tile_pool(name="ps_qk", bufs=1, space="PSUM"))
    ps_vt = ctx.enter_context(tc.tile_pool(name="ps_vt", bufs=1, space="PSUM"))
    ps_big = ctx.enter_context(tc.tile_pool(name="ps_big", bufs=2, space="PSUM"))
    ps_acc = ctx.enter_context(tc.tile_pool(name="ps_acc", bufs=2, space="PSUM"))

    # wqk gates the very first matmul: put it alone on the Scalar queue.
    # Remaining consts go on GpSimd ordered by first use; xt loads own Sync.
    wqk_sb = consts.tile([128, NCB, 128], BF16, tag="wqk")
    nc.scalar.dma_start(out=wqk_sb, in_=wqk_d.rearrange("(c p) h -> p c h", p=128))
    wv_sb = consts.tile([128, NCB, H], BF16, tag="wv")
    nc.gpsimd.dma_start(out=wv_sb, in_=wv_d.rearrange("(c p) h -> p c h", p=128))
    i64_sb = consts.tile([128, 64], BF16, tag="i64")
    nc.gpsimd.dma_start(out=i64_sb, in_=ident64_2)
    causal_sb = consts.tile([128, 128], BF16, tag="causal")
    nc.gpsimd.dma_start(out=causal_sb, in_=causal_d)
    i65_sb = consts.tile([65, 65], BF16, tag="i65")
    nc.gpsimd.dma_start(out=i65_sb, in_=ident65)

    qkT = persist.tile([128, T], BF16, tag="qkT")
    qT2 = persist.tile([128, T], BF16, tag="qT2")       # [qT; qT]
    kT2 = persist.tile([128, T // 2], BF16, tag="kT2")  # Tk pairs in halves
    vT = persist.tile([64, T], BF16, tag="vT")
    vT2 = persist.tile([128, T // 2], BF16, tag="vT2")  # odd Tk blocks, hi half
    v1 = persist.tile([128, NT, H + 1], BF16, tag="v1")  # [v | 1]
    nc.vector.memset(v1, 1.0)

    pending_av = None
    for j in range(NJ):
        jsl = slice(j * 512, (j + 1) * 512)

        # ---- projections for slice j --------------------------------
        xtj = []
        for c in range(NCB):
            xt = xts.tile([128, 512], BF16, tag="xt", name=f"xt{c}_{j}")
            eng = nc.sync if (j > 0 or c % 2 == 0) else nc.scalar
            eng.dma_start(out=xt, in_=xT_d[c * 128:(c + 1) * 128, jsl])
            xtj.append(xt)
        qk_ps = ps_qk.tile([128, 512], F32, tag="qkp", name=f"qk_ps{j}")
        for c in range(NCB):
            nc.tensor.matmul(qk_ps, lhsT=wqk_sb[:, c, :], rhs=xtj[c],
                             start=(c == 0), stop=(c == NCB - 1))
        nc.vector.tensor_copy(qkT[:, jsl], qk_ps)
        # qT into both halves; kT restacked into pair layout (on DVE's
        # queue: FIFO order after the copy above comes for free)
        nc.sync.dma_start(out=qT2[0:64, jsl], in_=qkT[0:64, jsl])
        nc.sync.dma_start(out=qT2[64:128, jsl], in_=qkT[0:64, jsl])
        for b in range(4):  # Tk block 4j+b -> half b%2, col block 2j+b//2
            half = (b % 2) * 64
            c0 = j * 256 + (b // 2) * 128
            nc.gpsimd.dma_start(
                out=kT2[half:half + 64, c0:c0 + 128],
                in_=qkT[64:128, j * 512 + b * 128:j * 512 + (b + 1) * 128])

        vT_ps = ps_vt.tile([64, 512], F32, tag="vtp", name=f"vT_ps{j}")
        for c in range(NCB):
            nc.tensor.matmul(vT_ps, lhsT=wv_sb[:, c, :], rhs=xtj[c],
                             start=(c == 0), stop=(c == NCB - 1))
        nc.vector.tensor_copy(vT[:, jsl], vT_ps)
        for bb in range(2):  # odd Tk blocks 4j+1, 4j+3 -> vT2 hi half
            tb = 4 * j + 2 * bb + 1
            c0 = (2 * j + bb) * 128
            nc.gpsimd.dma_start(
                out=vT2[64:128, c0:c0 + 128],
                in_=vT[:, tb * 128:(tb + 1) * 128])
        # v natural via row-packed identity matmuls (pair of Tk blocks)
        for mt in (2 * j, 2 * j + 1):
            tA, tB = 2 * mt, 2 * mt + 1
            vpA = ps_big.tile([128, H + 1], F32, tag="big", name=f"vpA{mt}")
            vpB = ps_big.tile([128, H + 1], F32, tag="big", name=f"vpB{mt}")
            nc.tensor.matmul(vpA[:, 0:H], lhsT=vT[:, tA * 128:(tA + 1) * 128],
                             rhs=i64_sb[0:64, :], start=True, stop=True)
            nc.tensor.matmul(vpB[:, 0:H],
                             lhsT=vT2[64:128, mt * 128:(mt + 1) * 128],
                             rhs=i64_sb[64:128, :], start=True, stop=True)
            nc.vector.tensor_copy(v1[:, tA, 0:H], vpA[:, 0:H])
            nc.vector.tensor_copy(v1[:, tB, 0:H], vpB[:, 0:H])

        # ---- deferred epilogue of slice j-1 -------------------------
        if pending_av is not None:
            emit_epilogue(nc, outts, outs, smalls, ps_acc, i65_sb, out_d,
                          *pending_av)
            pending_av = None

        # ---- attention for slice j (row-packed S^T, pipelined AV) ---
        av = ps_acc.tile([65, 512], F32, tag="accsm", name=f"av{j}")
        nblk = 4 * j + 4
        prev = None
        for m in range(2 * j + 2):
            sp2 = ps_big.tile([128, 1024], F32, tag="big", name=f"sp{j}_{m}")
            pt2 = pts.tile([128, 1024], BF16, tag="pt", name=f"pt{j}_{m}")
            n0s = []
            for half_idx, i in ((0, 2 * m), (1, 2 * m + 1)):
                g = i - 4 * j
                n0 = max(0, g) * 128
                p0 = half_idx * 64
                o = half_idx * 512
                nc.tensor.matmul(
                    sp2[:, o + n0:o + 512],
                    lhsT=kT2[p0:p0 + 64, m * 128:(m + 1) * 128],
                    rhs=qT2[p0:p0 + 64, j * 512 + n0:(j + 1) * 512],
                    start=True, stop=True)
                n0s.append(n0)
            if n0s[0] == 0 and n0s[1] == 0:  # one wide exp over both banks
                nc.scalar.activation(pt2, sp2,
                                     mybir.ActivationFunctionType.Exp,
                                     scale=SCALE)
            else:
                for half_idx in range(2):
                    o, n0 = half_idx * 512, n0s[half_idx]
                    nc.scalar.activation(
                        pt2[:, o + n0:o + 512], sp2[:, o + n0:o + 512],
                        mybir.ActivationFunctionType.Exp, scale=SCALE)
            for half_idx, i in ((0, 2 * m), (1, 2 * m + 1)):
                g = i - 4 * j
                if g >= 0:  # mask upper triangle of the diagonal block
                    o = half_idx * 512 + n0s[half_idx]
                    nc.vector.tensor_mul(
                        pt2[:, o:o + 128], pt2[:, o:o + 128], causal_sb)
            if prev is not None:
                emit_av(nc, av, v1, *prev, nblk)
            prev = (pt2, n0s, 2 * m)
        emit_av(nc, av, v1, *prev, nblk)
        pending_av = (av, j)

    emit_epilogue(nc, outts, outs, smalls, ps_acc, i65_sb, out_d, *pending_av)


def emit_av(nc, av, v1, pt2, n0s, i0, nblk):
    for d in range(2):
        i = i0 + d
        o, n0 = d * 512, n0s[d]
        nc.tensor.matmul(av[:, n0:512], lhsT=v1[:, i, :],
                         rhs=pt2[:, o + n0:o + 512],
                         start=(i == 0), stop=(i == nblk - 1))


def emit_epilogue(nc, outts, outs, smalls, ps_acc, i65_sb, out_d, av, j):
    osb = outts.tile([65, 512], BF16, tag="osb", name=f"osb{j}")
    nc.scalar.copy(osb, av)  # ScalarE: f32 PSUM -> bf16 SBUF
    for t in range(4):
        op = ps_acc.tile([128, H + 1], F32, tag="accsm", name=f"op{j}_{t}")
        nc.tensor.matmul(op, lhsT=osb[:, t * 128:(t + 1) * 128], rhs=i65_sb,
                         start=True, stop=True)
        rc = smalls.tile([128, 1], F32, tag="rc", name=f"rc{j}_{t}")
        nc.vector.reciprocal(rc, op[:, H:H + 1])
        ot = outs.tile([128, H], F32, tag="ot", name=f"ot{j}_{t}")
        nc.vector.tensor_scalar_mul(ot, op[:, 0:H], rc)
        r0 = (j * 4 + t) * 128
        eng = nc.scalar if t % 2 == 0 else nc.sync
        eng.dma_start(out=out_d[r0:r0 + 128, :], in_=ot)


_CACHED = {}


def _get_nc():
    if "nc" not in _CACHED:
        from contextlib import ExitStack
        nc = bacc.Bacc("TRN2", target_bir_lowering=False, debug=False,
                       num_devices=B)
        with tile.TileContext(nc) as tc:
            with ExitStack() as ctx:
                build_attention(nc, tc, ctx)
        nc.compile()
        _CACHED["nc"] = nc
    return _CACHED["nc"]


def kernel(inputs, Wq, Wk, Wv):
    inputs = np.asarray(inputs, dtype=np.float32)
    wqk = np.concatenate([np.asarray(Wq), np.asarray(Wk)], axis=1)
    wqk = np.ascontiguousarray(wqk).astype(npbf16)
    wv = np.ascontiguousarray(np.asarray(Wv)).astype(npbf16)

    in_maps = []
    for b in range(B):
        xT = np.ascontiguousarray(inputs[b].T).astype(npbf16)
        in_maps.append({"xT": xT, "wqk": wqk, "wv": wv})

    nc = _get_nc()
    res = run_bass_kernel_spmd(nc, in_maps, core_ids=list(range(B)))
    out = np.stack([res.results[b]["out"] for b in range(B)], axis=0)
    return out.astype(np.float32)


# revision 10
# speedup vs baseline: 1.5152x; 1.0526x over previous
"""Single-head causal attention on 8 TRN2 NeuronCores.

Problem shapes (hardcoded): B=8, T=2048, C=1024, H=64, fp32 I/O.
    q = x @ Wq; k = x @ Wk; v = x @ Wv          (per batch element)
    wei = softmax(causal_mask(q @ k.T * C**-0.5))
    out = wei @ v

Sharding: pure data parallel — one batch element per core, no collectives.

Per-core algorithm (matmuls bf16, fp32 PSUM accumulation):
  - host pre-transposes x -> xT [C, T] so C (the contraction dim of the
    QKV projections) lands on SBUF partitions; host packs [Wq|Wk].
  - per 512-wide Tq slice j: qkT = [Wq|Wk].T @ xT (qT rows 0:64, kT rows
    64:128); SBUF->SBUF DMAs build qT2 = [qT;qT] (both partition halves)
    and kT2 (Tk blocks 2m/2m+1 stacked in partition halves) so S^T
    matmuls run ROW-PACKED: two K=64 matmuls execute concurrently in the
    two 64-row halves of the PE array (row_grp packing).
  - vT = Wv.T @ xT; v natural recovered by row-packed identity matmuls,
    stored as v1 = [v | 1] (ones column -> softmax denominators free).
  - S^T pair tiles share one [128,1024] PSUM tensor (2 banks) so the
    exp runs as a single wide ScalarE ACTIVATE where possible.
    P^T = exp(S^T/32); no max subtraction needed (logits std ~0.25;
    softmax is shift invariant). Diagonal 128x128 blocks multiplied by a
    0/1 causal mask; fully-masked column ranges never computed
    (restricted-N matmuls).
  - [out|denom]^T accumulated via lhsT=v1 [128,65], rhs=P^T; epilogue
    transposes 128-col blocks back to natural layout with an identity
    matmul and normalizes per partition (reciprocal + tensor_scalar).
  - projections and attention interleave per j so the PE never idles
    (HAM clock gate stays released); slice j's epilogue is emitted after
    slice j+1's projections so ACT-dependent matmuls never stall the PE
    FIFO head; DMAs are spread across the Sync/GpSimd/Vector queues.
"""

import numpy as np
import ml_dtypes

import concourse.bass as bass
import concourse.mybir as mybir
import concourse.tile as tile
from concourse import bacc
from concourse.bass_utils import run_bass_kernel_spmd

B, T, C, H = 8, 2048, 1024, 64
NCB = C // 128          # 8 C-blocks
NT = T // 128           # 16 Tk-blocks of 128
NJ = T // 512           # 4 Tq-slices of 512
SCALE = float(C) ** -0.5  # 1/32

BF16 = mybir.dt.bfloat16
F32 = mybir.dt.float32
npbf16 = ml_dtypes.bfloat16


def build_attention(nc: bass.Bass, tc: tile.TileContext, ctx):
    xT_d = nc.dram_tensor("xT", [C, T], BF16, kind="ExternalInput").ap()
    wqk_d = nc.dram_tensor("wqk", [C, 128], BF16, kind="ExternalInput").ap()
    wv_d = nc.dram_tensor("wv", [C, H], BF16, kind="ExternalInput").ap()
    out_d = nc.dram_tensor("out", [T, H], F32, kind="ExternalOutput").ap()

    i64_2_np = np.concatenate([np.eye(64, dtype=npbf16)] * 2, axis=0)
    ident64_2 = nc.inline_tensor(i64_2_np, name="ident64_2").ap()
    ident65 = nc.inline_tensor(np.eye(65, dtype=npbf16), name="ident65").ap()
    causal_np = np.triu(np.ones((128, 128), dtype=npbf16))  # keep Tk<=Tq
    causal_d = nc.inline_tensor(causal_np, name="causal").ap()

    consts = ctx.enter_context(tc.tile_pool(name="consts", bufs=1))
    xts = ctx.enter_context(tc.tile_pool(name="xts", bufs=18))
    persist = ctx.enter_context(tc.tile_pool(name="persist", bufs=1))
    pts = ctx.enter_context(tc.tile_pool(name="pts", bufs=6))
    outts = ctx.enter_context(tc.tile_pool(name="outts", bufs=2))
    outs = ctx.enter_context(tc.tile_pool(name="outs", bufs=3))
    smalls = ctx.enter_context(tc.tile_pool(name="smalls", bufs=2))
    ps_qk = ctx.enter_context(tc.tile_pool(name="ps_qk", bufs=1, space="PSUM"))
    ps_vt = ctx.enter_context(tc.tile_pool(name="ps_vt", bufs=1, space="PSUM"))
    ps_big = ctx.enter_context(tc.tile_pool(name="ps_big", bufs=2, space="PSUM"))
    ps_acc = ctx.enter_context(tc.tile_pool(name="ps_acc", bufs=2, space="PSUM"))

    # wqk gates the very first matmul: put it alone on the Scalar queue.
    # Remaining consts go on GpSimd ordered by first use; xt loads own Sync.
    wqk_sb = consts.tile([128, NCB, 128], BF16, tag="wqk")
    nc.scalar.dma_start(out=wqk_sb, in_=wqk_d.rearrange("(c p) h -> p c h", p=128))
    wv_sb = consts.tile([128, NCB, H], BF16, tag="wv")
    nc.gpsimd.dma_start(out=wv_sb, in_=wv_d.rearrange("(c p) h -> p c h", p=128))
    i64_sb = consts.tile([128, 64], BF16, tag="i64")
    nc.gpsimd.dma_start(out=i64_sb, in_=ident64_2)
    causal_sb = consts.tile([128, 128], BF16, tag="causal")
    nc.gpsimd.dma_start(out=causal_sb, in_=causal_d)
    i65_sb = consts.tile([65, 65], BF16, tag="i65")
    nc.gpsimd.dma_start(out=i65_sb, in_=ident65)

    qkT = persist.tile([128, T], BF16, tag="qkT")
    qT2 = persist.tile([128, T], BF16, tag="qT2")       # [qT; qT]
    kT2 = persist.tile([128, T // 2], BF16, tag="kT2")  # Tk pairs in halves
    vT = persist.tile([64, T], BF16, tag="vT")
    vT2 = persist.tile([128, T // 2], BF16, tag="vT2")  # odd Tk blocks, hi half
    v1 = persist.tile([128, NT, H + 1], BF16, tag="v1")  # [v | 1]
    nc.vector.memset(v1, 1.0)

    pending_av = None
    for j in range(NJ):
        jsl = slice(j * 512, (j + 1) * 512)

        # ---- projections for slice j --------------------------------
        xtj = []
        for c in range(NCB):
            xt = xts.tile([128, 512], BF16, tag="xt", name=f"xt{c}_{j}")
            if j > 0:
                eng = nc.sync
            else:  # spread slice-0 loads over three queues for a fast start
                eng = (nc.sync, nc.scalar, nc.gpsimd)[c % 3]
            eng.dma_start(out=xt, in_=xT_d[c * 128:(c + 1) * 128, jsl])
            xtj.append(xt)
        qk_ps = ps_qk.tile([128, 512], F32, tag="qkp", name=f"qk_ps{j}")
        for c in range(NCB):
            nc.tensor.matmul(qk_ps, lhsT=wqk_sb[:, c, :], rhs=xtj[c],
                             start=(c == 0), stop=(c == NCB - 1))
        nc.vector.tensor_copy(qkT[:, jsl], qk_ps)
        # qT into both halves; kT restacked into pair layout (on DVE's
        # queue: FIFO order after the copy above comes for free)
        nc.sync.dma_start(out=qT2[0:64, jsl], in_=qkT[0:64, jsl])
        nc.sync.dma_start(out=qT2[64:128, jsl], in_=qkT[0:64, jsl])
        for b in range(4):  # Tk block 4j+b -> half b%2, col block 2j+b//2
            half = (b % 2) * 64
            c0 = j * 256 + (b // 2) * 128
            nc.gpsimd.dma_start(
                out=kT2[half:half + 64, c0:c0 + 128],
                in_=qkT[64:128, j * 512 + b * 128:j * 512 + (b + 1) * 128])

        vT_ps = ps_vt.tile([64, 512], F32, tag="vtp", name=f"vT_ps{j}")
        for c in range(NCB):
            nc.tensor.matmul(vT_ps, lhsT=wv_sb[:, c, :], rhs=xtj[c],
                             start=(c == 0), stop=(c == NCB - 1))
        nc.vector.tensor_copy(vT[:, jsl], vT_ps)
        for bb in range(2):  # odd Tk blocks 4j+1, 4j+3 -> vT2 hi half
            tb = 4 * j + 2 * bb + 1
            c0 = (2 * j + bb) * 128
            nc.gpsimd.dma_start(
                out=vT2[64:128, c0:c0 + 128],
                in_=vT[:, tb * 128:(tb + 1) * 128])
        # v natural via row-packed identity matmuls (pair of Tk blocks)
        for mt in (2 * j, 2 * j + 1):
            tA, tB = 2 * mt, 2 * mt + 1
            vpA = ps_big.tile([128, H + 1], F32, tag="big", name=f"vpA{mt}")
            vpB = ps_big.tile([128, H + 1], F32, tag="big", name=f"vpB{mt}")
            nc.tensor.matmul(vpA[:, 0:H], lhsT=vT[:, tA * 128:(tA + 1) * 128],
                             rhs=i64_sb[0:64, :], start=True, stop=True)
            nc.tensor.matmul(vpB[:, 0:H],
                             lhsT=vT2[64:128, mt * 128:(mt + 1) * 128],
                             rhs=i64_sb[64:128, :], start=True, stop=True)
            nc.vector.tensor_copy(v1[:, tA, 0:H], vpA[:, 0:H])
            nc.vector.tensor_copy(v1[:, tB, 0:H], vpB[:, 0:H])

        # ---- deferred epilogue of slice j-1 -------------------------
        if pending_av is not None:
            emit_epilogue(nc, outts, outs, smalls, ps_acc, i65_sb, out_d,
                          *pending_av)
            pending_av = None

        # ---- attention for slice j (row-packed S^T, pipelined AV) ---
        av = ps_acc.tile([65, 512], F32, tag="accsm", name=f"av{j}")
        nblk = 4 * j + 4
        prev = None
        for m in range(2 * j + 2):
            sp2 = ps_big.tile([128, 1024], F32, tag="big", name=f"sp{j}_{m}")
            pt2 = pts.tile([128, 1024], BF16, tag="pt", name=f"pt{j}_{m}")
            n0s = []
            for half_idx, i in ((0, 2 * m), (1, 2 * m + 1)):
                g = i - 4 * j
                n0 = max(0, g) * 128
                p0 = half_idx * 64
                o = half_idx * 512
                nc.tensor.matmul(
                    sp2[:, o + n0:o + 512],
                    lhsT=kT2[p0:p0 + 64, m * 128:(m + 1) * 128],
                    rhs=qT2[p0:p0 + 64, j * 512 + n0:(j + 1) * 512],
                    start=True, stop=True)
                n0s.append(n0)
            if n0s[0] == 0 and n0s[1] == 0:  # one wide exp over both banks
                nc.scalar.activation(pt2, sp2,
                                     mybir.ActivationFunctionType.Exp,
                                     scale=SCALE)
            else:
                for half_idx in range(2):
                    o, n0 = half_idx * 512, n0s[half_idx]
                    nc.scalar.activation(
                        pt2[:, o + n0:o + 512], sp2[:, o + n0:o + 512],
                        mybir.ActivationFunctionType.Exp, scale=SCALE)
            for half_idx, i in ((0, 2 * m), (1, 2 * m + 1)):
                g = i - 4 * j
                if g >= 0:  # mask upper triangle of the diagonal block
                    o = half_idx * 512 + n0s[half_idx]
                    nc.vector.tensor_mul(
                        pt2[:, o:o + 128], pt2[:, o:o + 128], causal_sb)
            if prev is not None:
                emit_av(nc, av, v1, *prev, nblk)
            prev = (pt2, n0s, 2 * m)
        emit_av(nc, av, v1, *prev, nblk)
        pending_av = (av, j)

    emit_epilogue(nc, outts, outs, smalls, ps_acc, i65_sb, out_d, *pending_av)


def emit_av(nc, av, v1, pt2, n0s, i0, nblk):
    for d in range(2):
        i = i0 + d
        o, n0 = d * 512, n0s[d]
        nc.tensor.matmul(av[:, n0:512], lhsT=v1[:, i, :],
                         rhs=pt2[:, o + n0:o + 512],
                         start=(i == 0), stop=(i == nblk - 1))


def emit_epilogue(nc, outts, outs, smalls, ps_acc, i65_sb, out_d, av, j):
    osb = outts.tile([65, 512], BF16, tag="osb", name=f"osb{j}")
    nc.vector.tensor_copy(osb, av)  # f32 PSUM -> bf16 SBUF
    for t in range(4):
        op = ps_acc.tile([128, H + 1], F32, tag="accsm", name=f"op{j}_{t}")
        nc.tensor.matmul(op, lhsT=osb[:, t * 128:(t + 1) * 128], rhs=i65_sb,
                         start=True, stop=True)
        rc = smalls.tile([128, 1], F32, tag="rc", name=f"rc{j}_{t}")
        nc.vector.reciprocal(rc, op[:, H:H + 1])
        ot = outs.tile([128, H], F32, tag="ot", name=f"ot{j}_{t}")
        nc.vector.tensor_scalar_mul(ot, op[:, 0:H], rc)
        r0 = (j * 4 + t) * 128
        eng = nc.gpsimd if t % 2 == 0 else nc.sync
        eng.dma_start(out=out_d[r0:r0 + 128, :], in_=ot)


_CACHED = {}


def _get_nc():
    if "nc" not in _CACHED:
        from contextlib import ExitStack
        nc = bacc.Bacc("TRN2", target_bir_lowering=False, debug=False,
                       num_devices=B)
        with tile.TileContext(nc) as tc:
            with ExitStack() as ctx:
                build_attention(nc, tc, ctx)
        nc.compile()
        _CACHED["nc"] = nc
    return _CACHED["nc"]


def kernel(inputs, Wq, Wk, Wv):
    inputs = np.asarray(inputs, dtype=np.float32)
    wqk = np.concatenate([np.asarray(Wq), np.asarray(Wk)], axis=1)
    wqk = np.ascontiguousarray(wqk).astype(npbf16)
    wv = np.ascontiguousarray(np.asarray(Wv)).astype(npbf16)

    in_maps = []
    for b in range(B):
        xT = np.ascontiguousarray(inputs[b].T).astype(npbf16)
        in_maps.append({"xT": xT, "wqk": wqk, "wv": wv})

    nc = _get_nc()
    res = run_bass_kernel_spmd(nc, in_maps, core_ids=list(range(B)))
    out = np.stack([res.results[b]["out"] for b in range(B)], axis=0)
    return out.astype(np.float32)
